# revision 7
# baseline (speedup 1.0000x reference)
"""IrrepsLinear Trainium2 kernel: y = per-irrep-block x @ W / sqrt(mul).

Irreps layout: 256x0e + 128x1o + 64x2e -> blocks of width 256*1, 128*3, 64*5.
Data-parallel over 8 NeuronCores: each core gets 12500 nodes.

v14 strategy (hybrid int8/fp16 in, int8 out, double-buffered PSUM):
  - All seven 128-row output groups of a slice live in ONE PSUM tile
    [128, 8, 256] (4 banks; group 7 holds the paired m4 output), allocated
    with bufs=2 (8 banks total). Two slices in flight decouple the PE from
    evac latency: a matmul only WARs on the evac two slices back.
  - The PE sustains ~2.24 GHz only when matmuls issue near back-to-back;
    a couple of discardable pad matmuls into group 7 at each slice start
    absorb residual dependency latency without extra PSUM.
  - Output y rides DRAM as int8 (per-tensor scale, clip 4 sigma); evac is
    a plain fp32->int8 copy (HW rounds-to-nearest-even and saturates).
  - Input x: first 512 permuted features (block0 + block1 m0,m1) ride as
    int8, dequantized on-chip to fp16 in window-sized chunks (ACT: g0,g1;
    GPSIMD: g2,g3) interleaved between slice evacs; remaining 448 features
    ride fp16 and feed the PE directly. Scales fold into the weights.
  - Block2's five 64-wide m-components run at full PE width: (m0,m1) and
    (m2,m3) pair into 128 partitions with a block-diagonal W2; m4 of the
    window's first/second node-halves pair the same way (xbp/ybp tensors).
  - Evac split ACT (groups 0:3) / DVE (groups 3:7 + m4); loads on the SP
    HWDGE ring, stores on the ACT ring; windows [1024,3072x3,2048,212]
    keep the head fill and tail drain short.
"""

import numpy as np

NCORES = 8
N_TOTAL = 100000
NSH = N_TOTAL // NCORES   # 12500 nodes per core
D = 960
MMW = 256                 # matmul slice width (half a fp32 PSUM bank)
PAD = 2                   # discardable pad matmuls per slice (DVFS keep-alive)
DEQC = 1024               # dequant chunk width (ACT); GPSIMD uses DEQC//2

WINDOWS = [1024, 3072, 3072, 3072, 2048, 212]
assert sum(WINDOWS) == NSH and all(w % 2 == 0 for w in WINDOWS)
OFFS = np.concatenate([[0], np.cumsum(WINDOWS)[:-1]]).tolist()

CLIP_X = 4.0
CLIP_Y = 4.0
S_X = CLIP_X / 127.0
S_Y = CLIP_Y / 127.0

_BUILD_CACHE = {}


def _perm():
    p = list(range(256))
    for m in range(3):
        p += [256 + 3 * i + m for i in range(128)]
    for m in range(5):
        p += [640 + 5 * i + m for i in range(64)]
    return np.asarray(p, dtype=np.int64)

_PERM = _perm()


def _build_program():
    import concourse.bass as bass  # noqa: F401
    import concourse.bacc as bacc
    import concourse.mybir as mybir
    import concourse.tile as tile

    key = (MMW, tuple(WINDOWS), PAD, DEQC, "v14")
    if key in _BUILD_CACHE:
        return _BUILD_CACHE[key]

    i8 = mybir.dt.int8
    f16 = mybir.dt.float16
    f32 = mybir.dt.float32

    nc = bacc.Bacc(
        "TRN2", target_bir_lowering=False, debug=False, enable_asserts=False
    )
    xa8 = nc.dram_tensor("xa8", [128, 4 * NSH], i8, kind="ExternalInput").ap()
    xa16 = nc.dram_tensor("xa16", [128, 3 * NSH], f16, kind="ExternalInput").ap()
    xbp = nc.dram_tensor("xbp", [128, NSH // 2], f16, kind="ExternalInput").ap()
    w0 = nc.dram_tensor("w0", [256, 256], f16, kind="ExternalInput").ap()
    w1q = nc.dram_tensor("w1q", [128, 128], f16, kind="ExternalInput").ap()
    w1f = nc.dram_tensor("w1f", [128, 128], f16, kind="ExternalInput").ap()
    w2d = nc.dram_tensor("w2d", [128, 128], f16, kind="ExternalInput").ap()
    ya = nc.dram_tensor("ya", [128, 7 * NSH], i8, kind="ExternalOutput").ap()
    ybp = nc.dram_tensor("ybp", [128, NSH // 2], i8, kind="ExternalOutput").ap()

    with tile.TileContext(nc) as tc:
        with (
            tc.tile_pool(name="sb", bufs=1) as spool,
            tc.tile_pool(name="ps", bufs=2, space="PSUM") as pspool,
        ):
            w0t0 = spool.tile([128, 256], f16, name="w0t0", tag="w0t0")
            nc.sync.dma_start(w0t0[:], w0[0:128, :])
            w0t1 = spool.tile([128, 256], f16, name="w0t1", tag="w0t1")
            nc.sync.dma_start(w0t1[:], w0[128:256, :])
            w1qt = spool.tile([128, 128], f16, name="w1qt", tag="w1qt")
            nc.sync.dma_start(w1qt[:], w1q[:, :])
            w1ft = spool.tile([128, 128], f16, name="w1ft", tag="w1ft")
            nc.sync.dma_start(w1ft[:], w1f[:, :])
            w2dt = spool.tile([128, 128], f16, name="w2dt", tag="w2dt")
            nc.sync.dma_start(w2dt[:], w2d[:, :])

            # Per-window tiles + per-slice work items (flat across windows)
            items = []       # one entry per matmul slice
            deq_work = []    # (slice_idx_to_emit_after, fn) dequant chunks
            for wi, (c0, sw) in enumerate(zip(OFFS, WINDOWS)):
                h = sw // 2
                xat8 = spool.tile([128, 4, sw], i8, name=f"xa8_{wi}",
                                  tag="xa8", bufs=2)
                nc.sync.dma_start(xat8[:], xa8[:, 4 * c0 : 4 * (c0 + sw)])
                xat16 = spool.tile([128, 3, sw], f16, name=f"xa16_{wi}",
                                   tag="xa16", bufs=2)
                nc.sync.dma_start(xat16[:], xa16[:, 3 * c0 : 3 * (c0 + sw)])
                xbt = spool.tile([128, h], f16, name=f"xb{wi}", tag="xb",
                                 bufs=2)
                nc.sync.dma_start(xbt[:], xbp[:, c0 // 2 : c0 // 2 + h])
                yat = spool.tile([128, 7, sw], i8, name=f"ya{wi}", tag="ya",
                                 bufs=2)
                ybt = spool.tile([128, h], i8, name=f"yb{wi}", tag="yb",
                                 bufs=2)
                xf = spool.tile([128, 4, sw], f16, name=f"xf{wi}", tag="xf",
                                bufs=2)

                base = len(items)
                slices = [
                    (i * MMW, min((i + 1) * MMW, sw))
                    for i in range((sw + MMW - 1) // MMW)
                ]
                m4s = [
                    (i * MMW, min((i + 1) * MMW, h))
                    for i in range((h + MMW - 1) // MMW)
                ]
                m4i = 0
                for si, (lo, hi) in enumerate(slices):
                    m4 = None
                    if (si % 2 == 1 or si == len(slices) - 1) and m4i < len(m4s):
                        m4 = m4s[m4i]
                        m4i += 1
                    last = si == len(slices) - 1
                    stores = (c0, sw, h) if last else None
                    items.append((lo, hi, xf, xat16, xbt, yat, ybt, m4, stores))
                assert m4i == len(m4s)

                # dequant chunks: ACT does g0,g1; GPSIMD does g2,g3 (half
                # chunk size). Schedule chunk c after enough earlier slices.
                achunks = [
                    (i * DEQC, min((i + 1) * DEQC, sw))
                    for i in range((sw + DEQC - 1) // DEQC)
                ]
                gw = DEQC // 2
                gchunks = [
                    (i * gw, min((i + 1) * gw, sw))
                    for i in range((sw + gw - 1) // gw)
                ]

                def mk_act(xat8=xat8, xf=xf):
                    def f(lo, hi):
                        nc.scalar.copy(xf[:, 0:2, lo:hi], xat8[:, 0:2, lo:hi])
                    return f

                def mk_gp(xat8=xat8, xf=xf):
                    def f(lo, hi):
                        nc.gpsimd.tensor_copy(xf[:, 2:4, lo:hi],
                                              xat8[:, 2:4, lo:hi])
                    return f

                fa, fg = mk_act(), mk_gp()
                na, ng = len(achunks), len(gchunks)
                ns = len(slices)
                for ci, (qlo, qhi) in enumerate(achunks):
                    emit_after = base + (ci * ns) // na - 1
                    deq_work.append((emit_after, fa, qlo, qhi))
                for ci, (qlo, qhi) in enumerate(gchunks):
                    emit_after = base + (ci * ns) // ng - 1
                    deq_work.append((emit_after, fg, qlo, qhi))

            deq_by_slot = {}
            for emit_after, f, qlo, qhi in deq_work:
                # pipeline: emit each chunk PIPE slices before first use
                PIPE = 4
                slot = emit_after - PIPE
                deq_by_slot.setdefault(slot, []).append((f, qlo, qhi))

            def emit_deq(slot):
                for f, qlo, qhi in deq_by_slot.pop(slot, ()):
                    f(qlo, qhi)

            def mm_evac(k):
                lo, hi, xf, xat16, xbt, yat, ybt, m4, stores = items[k]
                n = hi - lo

                ps = pspool.tile([128, 8, MMW], f32, name=f"ps_{k}", tag="ps")

                # pad matmuls into group 7 (overwritten by m4 below if any)
                for p in range(PAD):
                    nc.tensor.matmul(
                        ps[:, 7, 0:n], w2dt[:], xat16[:, p % 3, lo:hi],
                        start=True, stop=True,
                    )

                # block0: 256x256 = 2 out-blocks x 2 K-groups
                for ob in range(2):
                    oc = slice(128 * ob, 128 * (ob + 1))
                    nc.tensor.matmul(
                        ps[:, ob, 0:n], w0t0[:, oc], xf[:, 0, lo:hi],
                        start=True, stop=False,
                    )
                    nc.tensor.matmul(
                        ps[:, ob, 0:n], w0t1[:, oc], xf[:, 1, lo:hi],
                        start=False, stop=True,
                    )
                # block1: m0,m1 (int8 path), m2 (fp16 direct)
                for m in range(2):
                    nc.tensor.matmul(
                        ps[:, 2 + m, 0:n], w1qt[:], xf[:, 2 + m, lo:hi],
                        start=True, stop=True,
                    )
                nc.tensor.matmul(
                    ps[:, 4, 0:n], w1ft[:], xat16[:, 0, lo:hi],
                    start=True, stop=True,
                )
                # block2 m0..m3 pairs (fp16 direct)
                for g in range(2):
                    nc.tensor.matmul(
                        ps[:, 5 + g, 0:n], w2dt[:], xat16[:, 1 + g, lo:hi],
                        start=True, stop=True,
                    )
                # block2 m4, node-half-paired (fp16 direct), group 7
                if m4 is not None:
                    ko, khi = m4
                    nk = khi - ko
                    nc.tensor.matmul(
                        ps[:, 7, 0:nk], w2dt[:], xbt[:, ko:khi],
                        start=True, stop=True,
                    )

                nc.scalar.copy(yat[:, 0:3, lo:hi], ps[:, 0:3, 0:n])
                nc.vector.tensor_copy(yat[:, 3:7, lo:hi], ps[:, 3:7, 0:n])
                if m4 is not None:
                    ko, khi = m4
                    nc.vector.tensor_copy(ybt[:, ko:khi], ps[:, 7, 0 : khi - ko])

                if stores is not None:
                    c0, sw, h = stores
                    nc.scalar.dma_start(ya[:, 7 * c0 : 7 * (c0 + sw)], yat[:])
                    nc.scalar.dma_start(ybp[:, c0 // 2 : c0 // 2 + h], ybt[:])

            NK = len(items)
            for s in sorted(k for k in deq_by_slot if k < 0):
                emit_deq(s)
            for k in range(NK):
                emit_deq(k)
                mm_evac(k)

    nc.compile()
    _BUILD_CACHE[key] = nc
    return nc


TRACE = False
LAST_RESULT = None


def kernel(x, W0, W1, W2):
    from concourse import bass_utils

    nc = _build_program()

    # weights: fold 1/sqrt(mul), input scale (int8 paths) and output scale
    w0s = (np.asarray(W0, np.float32) * (S_X / (16.0 * S_Y))).astype(np.float16)
    w1qs = (np.asarray(W1, np.float32) * (S_X / (np.sqrt(128.0) * S_Y))
            ).astype(np.float16)
    w1fs = (np.asarray(W1, np.float32) * (1.0 / (np.sqrt(128.0) * S_Y))
            ).astype(np.float16)
    w2 = (np.asarray(W2, np.float32) * (1.0 / (8.0 * S_Y))).astype(np.float16)
    w2dv = np.zeros((128, 128), dtype=np.float16)
    w2dv[0:64, 0:64] = w2
    w2dv[64:128, 64:128] = w2

    A = np.asarray(x)[:, _PERM].reshape(NCORES, NSH, D)
    A8 = np.clip(np.rint(A[:, :, :512] * (1.0 / S_X)), -127, 127
                 ).astype(np.int8)
    A16 = A[:, :, 512:896].astype(np.float16)
    AB = A[:, :, 896:].astype(np.float16)

    blocks8, blocks16, bblocks = [], [], []
    for c0, sw in zip(OFFS, WINDOWS):
        b8 = A8[:, c0 : c0 + sw].reshape(NCORES, sw, 4, 128)
        blocks8.append(b8.transpose(0, 3, 2, 1).reshape(NCORES, 128, 4 * sw))
        b16 = A16[:, c0 : c0 + sw].reshape(NCORES, sw, 3, 128)
        blocks16.append(b16.transpose(0, 3, 2, 1).reshape(NCORES, 128, 3 * sw))
        F = AB[:, c0 : c0 + sw]                          # [C, sw, 64]
        h = sw // 2
        bblocks.append(np.concatenate(
            [F[:, :h].transpose(0, 2, 1), F[:, h:].transpose(0, 2, 1)], axis=1
        ))                                               # [C, 128, h]
    xa8_all = np.ascontiguousarray(np.concatenate(blocks8, axis=2))
    xa16_all = np.ascontiguousarray(np.concatenate(blocks16, axis=2))
    xb_all = np.ascontiguousarray(np.concatenate(bblocks, axis=2))

    in_maps = []
    for c in range(NCORES):
        in_maps.append({
            "xa8": xa8_all[c], "xa16": xa16_all[c], "xbp": xb_all[c],
            "w0": w0s, "w1q": w1qs, "w1f": w1fs, "w2d": w2dv,
        })

    res = bass_utils.run_bass_kernel_spmd(
        nc, in_maps, core_ids=list(range(NCORES)), trace=TRACE
    )
    global LAST_RESULT
    LAST_RESULT = res

    out = np.empty((N_TOTAL, D), dtype=np.float32)
    Yp = np.empty((NCORES, NSH, D), dtype=np.float32)
    for c in range(NCORES):
        yac = res.results[c]["ya"]    # [128, 7*NSH] int8
        ybc = res.results[c]["ybp"]   # [128, NSH//2] int8
        for c0, sw in zip(OFFS, WINDOWS):
            blk = yac[:, 7 * c0 : 7 * (c0 + sw)].reshape(128, 7, sw)
            Yp[c, c0 : c0 + sw, :896] = (
                blk.transpose(2, 1, 0).reshape(sw, 896).astype(np.float32)
            )
            h = sw // 2
            yb = ybc[:, c0 // 2 : c0 // 2 + h]
            Yp[c, c0 : c0 + h, 896:] = yb[0:64].T.astype(np.float32)
            Yp[c, c0 + h : c0 + sw, 896:] = yb[64:128].T.astype(np.float32)
    Yp *= S_Y
    out[:, _PERM] = Yp.reshape(N_TOTAL, D)
    return out


# revision 8
# speedup vs baseline: 1.0461x; 1.0461x over previous
"""IrrepsLinear Trainium2 kernel: y = per-irrep-block x @ W / sqrt(mul).

Irreps layout: 256x0e + 128x1o + 64x2e -> blocks of width 256*1, 128*3, 64*5.
Data-parallel over 8 NeuronCores: each core gets 12500 nodes.

v15 strategy (hybrid int8/fp16 in, int8 out, latency-tuned at MMW=512):
  - MMW=512 keeps LDWEIGHTS hidden behind matmuls (at 256 it serializes
    and halves PE throughput). The PE sustains ~2.24 GHz when matmuls
    issue near back-to-back; 3 discardable pad matmuls at each slice
    start (written into ps0's bank, overwritten by the real block0 group)
    absorb cross-engine dependency latency without costing a PSUM bank.
  - PSUM: ps0 [128,2,512] (block0), ps1 [128,3,512] (block1), ps2
    [128,3,512] (block2 pairs + paired m4) = 8 banks, one slice in
    flight; evacs are emitted engine-first in each round (b0 earliest)
    so the next slice's WAR clears early.
  - Output y rides DRAM as int8 (per-tensor scale, clip 4 sigma); evac is
    a plain fp32->int8 copy (HW rounds-to-nearest-even and saturates).
  - Input x: first 512 permuted features (block0 + block1 m0,m1) ride as
    int8, dequantized on-chip to fp16 (exact integers; scales fold into
    the fp16 weights): ACT g0,g1; DVE half of g2; GPSIMD g2-half + g3.
    Remaining 448 features ride fp16 and feed the PE directly. This
    balances DMA bytes (345 GB/s measured) against copy-engine rates
    (ACT ~0.86, DVE ~0.78, GPSIMD ~0.27 elem/ns measured).
  - Block2's five 64-wide m-components run at full PE width: (m0,m1) and
    (m2,m3) pair into 128 partitions with a block-diagonal W2; m4 of the
    window's first/second node-halves pair the same way (xbp/ybp tensors).
  - Dequant runs two slices ahead; loads on the SP HWDGE ring, stores on
    the ACT ring; windows [1024,3072x3,2048,212] keep head/tail short.
"""

import numpy as np

NCORES = 8
N_TOTAL = 100000
NSH = N_TOTAL // NCORES   # 12500 nodes per core
D = 960
MMW = 512                 # matmul slice width (= one fp32 PSUM bank)
PAD = 3                   # discardable pad matmuls per slice (DVFS keep-alive)

WINDOWS = [1024, 3072, 3072, 3072, 2048, 212]
assert sum(WINDOWS) == NSH and all(w % 2 == 0 for w in WINDOWS)
OFFS = np.concatenate([[0], np.cumsum(WINDOWS)[:-1]]).tolist()

CLIP_X = 4.0
CLIP_Y = 4.0
S_X = CLIP_X / 127.0
S_Y = CLIP_Y / 127.0

_BUILD_CACHE = {}


def _perm():
    p = list(range(256))
    for m in range(3):
        p += [256 + 3 * i + m for i in range(128)]
    for m in range(5):
        p += [640 + 5 * i + m for i in range(64)]
    return np.asarray(p, dtype=np.int64)

_PERM = _perm()


def _build_program():
    import concourse.bass as bass  # noqa: F401
    import concourse.bacc as bacc
    import concourse.mybir as mybir
    import concourse.tile as tile

    key = (MMW, tuple(WINDOWS), PAD, "v15")
    if key in _BUILD_CACHE:
        return _BUILD_CACHE[key]

    i8 = mybir.dt.int8
    f16 = mybir.dt.float16
    f32 = mybir.dt.float32

    nc = bacc.Bacc(
        "TRN2", target_bir_lowering=False, debug=False, enable_asserts=False
    )
    xa8 = nc.dram_tensor("xa8", [128, 4 * NSH], i8, kind="ExternalInput").ap()
    xa16 = nc.dram_tensor("xa16", [128, 3 * NSH], f16, kind="ExternalInput").ap()
    xbp = nc.dram_tensor("xbp", [128, NSH // 2], f16, kind="ExternalInput").ap()
    w0 = nc.dram_tensor("w0", [256, 256], f16, kind="ExternalInput").ap()
    w1q = nc.dram_tensor("w1q", [128, 128], f16, kind="ExternalInput").ap()
    w1f = nc.dram_tensor("w1f", [128, 128], f16, kind="ExternalInput").ap()
    w2d = nc.dram_tensor("w2d", [128, 128], f16, kind="ExternalInput").ap()
    ya = nc.dram_tensor("ya", [128, 7 * NSH], i8, kind="ExternalOutput").ap()
    ybp = nc.dram_tensor("ybp", [128, NSH // 2], i8, kind="ExternalOutput").ap()

    with tile.TileContext(nc) as tc:
        with (
            tc.tile_pool(name="sb", bufs=1) as spool,
            tc.tile_pool(name="ps", bufs=1, space="PSUM") as pspool,
        ):
            w0t0 = spool.tile([128, 256], f16, name="w0t0", tag="w0t0")
            nc.sync.dma_start(w0t0[:], w0[0:128, :])
            w0t1 = spool.tile([128, 256], f16, name="w0t1", tag="w0t1")
            nc.sync.dma_start(w0t1[:], w0[128:256, :])
            w1qt = spool.tile([128, 128], f16, name="w1qt", tag="w1qt")
            nc.sync.dma_start(w1qt[:], w1q[:, :])
            w1ft = spool.tile([128, 128], f16, name="w1ft", tag="w1ft")
            nc.sync.dma_start(w1ft[:], w1f[:, :])
            w2dt = spool.tile([128, 128], f16, name="w2dt", tag="w2dt")
            nc.sync.dma_start(w2dt[:], w2d[:, :])

            items = []
            for wi, (c0, sw) in enumerate(zip(OFFS, WINDOWS)):
                h = sw // 2
                xat8 = spool.tile([128, 4, sw], i8, name=f"xa8_{wi}",
                                  tag="xa8", bufs=2)
                nc.sync.dma_start(xat8[:], xa8[:, 4 * c0 : 4 * (c0 + sw)])
                xat16 = spool.tile([128, 3, sw], f16, name=f"xa16_{wi}",
                                   tag="xa16", bufs=2)
                nc.sync.dma_start(xat16[:], xa16[:, 3 * c0 : 3 * (c0 + sw)])
                xbt = spool.tile([128, h], f16, name=f"xb{wi}", tag="xb",
                                 bufs=2)
                nc.sync.dma_start(xbt[:], xbp[:, c0 // 2 : c0 // 2 + h])
                yat = spool.tile([128, 7, sw], i8, name=f"ya{wi}", tag="ya",
                                 bufs=2)
                ybt = spool.tile([128, h], i8, name=f"yb{wi}", tag="yb",
                                 bufs=2)

                slices = [
                    (i * MMW, min((i + 1) * MMW, sw))
                    for i in range((sw + MMW - 1) // MMW)
                ]
                m4s = [
                    (i * MMW, min((i + 1) * MMW, h))
                    for i in range((h + MMW - 1) // MMW)
                ]
                m4i = 0
                for si, (lo, hi) in enumerate(slices):
                    m4 = None
                    if (si % 2 == 1 or si == len(slices) - 1) and m4i < len(m4s):
                        m4 = m4s[m4i]
                        m4i += 1
                    last = si == len(slices) - 1
                    stores = (c0, sw, h) if last else None
                    items.append((lo, hi, xat8, xat16, xbt, yat, ybt, m4,
                                  stores))
                assert m4i == len(m4s)

            xfs = {}

            def deq(k):
                lo, hi, xat8, _, _, _, _, _, _ = items[k]
                n = hi - lo
                c = min(256, n)
                xf = spool.tile([128, 4, MMW], f16, name=f"xf{k}", tag="xf",
                                bufs=3)
                xfs[k] = xf
                nc.scalar.copy(xf[:, 0:2, 0:n], xat8[:, 0:2, lo:hi])
                nc.gpsimd.tensor_copy(xf[:, 3:4, 0:n], xat8[:, 3:4, lo:hi])
                nc.gpsimd.tensor_copy(xf[:, 2, 0:c], xat8[:, 2, lo : lo + c])
                if n > c:
                    nc.vector.tensor_copy(xf[:, 2, c:n], xat8[:, 2, lo + c : hi])

            def mm_evac(k):
                lo, hi, xat8, xat16, xbt, yat, ybt, m4, stores = items[k]
                n = hi - lo
                xf = xfs.pop(k)

                ps0 = pspool.tile([128, 2, MMW], f32, name=f"ps0_{k}",
                                  tag="ps0")
                ps1 = pspool.tile([128, 3, MMW], f32, name=f"ps1_{k}",
                                  tag="ps1")
                ps2 = pspool.tile([128, 3, MMW], f32, name=f"ps2_{k}",
                                  tag="ps2")

                # pads: overwritten by block0's first group below (WAW on PE)
                for p in range(PAD):
                    nc.tensor.matmul(
                        ps0[:, 0, 0:n], w2dt[:], xat16[:, p % 3, lo:hi],
                        start=True, stop=True,
                    )

                # block0: 256x256 = 2 out-blocks x 2 K-groups
                for ob in range(2):
                    oc = slice(128 * ob, 128 * (ob + 1))
                    nc.tensor.matmul(
                        ps0[:, ob, 0:n], w0t0[:, oc], xf[:, 0, 0:n],
                        start=True, stop=False,
                    )
                    nc.tensor.matmul(
                        ps0[:, ob, 0:n], w0t1[:, oc], xf[:, 1, 0:n],
                        start=False, stop=True,
                    )
                nc.scalar.copy(yat[:, 0:2, lo:hi], ps0[:, :, 0:n])

                # block1: m0,m1 (int8 path), m2 (fp16 direct)
                for m in range(2):
                    nc.tensor.matmul(
                        ps1[:, m, 0:n], w1qt[:], xf[:, 2 + m, 0:n],
                        start=True, stop=True,
                    )
                nc.tensor.matmul(
                    ps1[:, 2, 0:n], w1ft[:], xat16[:, 0, lo:hi],
                    start=True, stop=True,
                )
                nc.vector.tensor_copy(yat[:, 2:5, lo:hi], ps1[:, :, 0:n])

                # block2 (m0,m1),(m2,m3) pairs + paired m4 (all fp16 direct)
                for g in range(2):
                    nc.tensor.matmul(
                        ps2[:, g, 0:n], w2dt[:], xat16[:, 1 + g, lo:hi],
                        start=True, stop=True,
                    )
                if m4 is not None:
                    ko, khi = m4
                    nc.tensor.matmul(
                        ps2[:, 2, 0 : khi - ko], w2dt[:], xbt[:, ko:khi],
                        start=True, stop=True,
                    )
                nc.scalar.copy(yat[:, 5:6, lo:hi], ps2[:, 0:1, 0:n])
                nc.vector.tensor_copy(yat[:, 6:7, lo:hi], ps2[:, 1:2, 0:n])
                if m4 is not None:
                    ko, khi = m4
                    nc.scalar.copy(ybt[:, ko:khi], ps2[:, 2, 0 : khi - ko])

                if stores is not None:
                    c0, sw, h = stores
                    nc.scalar.dma_start(ya[:, 7 * c0 : 7 * (c0 + sw)], yat[:])
                    nc.scalar.dma_start(ybp[:, c0 // 2 : c0 // 2 + h], ybt[:])

            NK = len(items)
            deq(0)
            deq(1)
            for k in range(NK):
                mm_evac(k)
                if k + 2 < NK:
                    deq(k + 2)

    nc.compile()
    _BUILD_CACHE[key] = nc
    return nc


TRACE = False
LAST_RESULT = None


def kernel(x, W0, W1, W2):
    from concourse import bass_utils

    nc = _build_program()

    # weights: fold 1/sqrt(mul), input scale (int8 paths) and output scale
    w0s = (np.asarray(W0, np.float32) * (S_X / (16.0 * S_Y))).astype(np.float16)
    w1qs = (np.asarray(W1, np.float32) * (S_X / (np.sqrt(128.0) * S_Y))
            ).astype(np.float16)
    w1fs = (np.asarray(W1, np.float32) * (1.0 / (np.sqrt(128.0) * S_Y))
            ).astype(np.float16)
    w2 = (np.asarray(W2, np.float32) * (1.0 / (8.0 * S_Y))).astype(np.float16)
    w2dv = np.zeros((128, 128), dtype=np.float16)
    w2dv[0:64, 0:64] = w2
    w2dv[64:128, 64:128] = w2

    A = np.asarray(x)[:, _PERM].reshape(NCORES, NSH, D)
    A8 = np.clip(np.rint(A[:, :, :512] * (1.0 / S_X)), -127, 127
                 ).astype(np.int8)
    A16 = A[:, :, 512:896].astype(np.float16)
    AB = A[:, :, 896:].astype(np.float16)

    blocks8, blocks16, bblocks = [], [], []
    for c0, sw in zip(OFFS, WINDOWS):
        b8 = A8[:, c0 : c0 + sw].reshape(NCORES, sw, 4, 128)
        blocks8.append(b8.transpose(0, 3, 2, 1).reshape(NCORES, 128, 4 * sw))
        b16 = A16[:, c0 : c0 + sw].reshape(NCORES, sw, 3, 128)
        blocks16.append(b16.transpose(0, 3, 2, 1).reshape(NCORES, 128, 3 * sw))
        F = AB[:, c0 : c0 + sw]                          # [C, sw, 64]
        h = sw // 2
        bblocks.append(np.concatenate(
            [F[:, :h].transpose(0, 2, 1), F[:, h:].transpose(0, 2, 1)], axis=1
        ))                                               # [C, 128, h]
    xa8_all = np.ascontiguousarray(np.concatenate(blocks8, axis=2))
    xa16_all = np.ascontiguousarray(np.concatenate(blocks16, axis=2))
    xb_all = np.ascontiguousarray(np.concatenate(bblocks, axis=2))

    in_maps = []
    for c in range(NCORES):
        in_maps.append({
            "xa8": xa8_all[c], "xa16": xa16_all[c], "xbp": xb_all[c],
            "w0": w0s, "w1q": w1qs, "w1f": w1fs, "w2d": w2dv,
        })

    res = bass_utils.run_bass_kernel_spmd(
        nc, in_maps, core_ids=list(range(NCORES)), trace=TRACE
    )
    global LAST_RESULT
    LAST_RESULT = res

    out = np.empty((N_TOTAL, D), dtype=np.float32)
    Yp = np.empty((NCORES, NSH, D), dtype=np.float32)
    for c in range(NCORES):
        yac = res.results[c]["ya"]    # [128, 7*NSH] int8
        ybc = res.results[c]["ybp"]   # [128, NSH//2] int8
        for c0, sw in zip(OFFS, WINDOWS):
            blk = yac[:, 7 * c0 : 7 * (c0 + sw)].reshape(128, 7, sw)
            Yp[c, c0 : c0 + sw, :896] = (
                blk.transpose(2, 1, 0).reshape(sw, 896).astype(np.float32)
            )
            h = sw // 2
            yb = ybc[:, c0 // 2 : c0 // 2 + h]
            Yp[c, c0 : c0 + h, 896:] = yb[0:64].T.astype(np.float32)
            Yp[c, c0 + h : c0 + sw, 896:] = yb[64:128].T.astype(np.float32)
    Yp *= S_Y
    out[:, _PERM] = Yp.reshape(N_TOTAL, D)
    return out


# revision 9
# speedup vs baseline: 1.2481x; 1.1931x over previous
"""IrrepsLinear Trainium2 kernel: y = per-irrep-block x @ W / sqrt(mul).

Irreps layout: 256x0e + 128x1o + 64x2e -> blocks of width 256*1, 128*3, 64*5.
Data-parallel over 8 NeuronCores: each core gets 12500 nodes.

v15 strategy (hybrid int8/fp16 in, int8 out, latency-tuned at MMW=512):
  - MMW=512 keeps LDWEIGHTS hidden behind matmuls (at 256 it serializes
    and halves PE throughput). The PE sustains ~2.24 GHz when matmuls
    issue near back-to-back; 3 discardable pad matmuls at each slice
    start (written into ps0's bank, overwritten by the real block0 group)
    absorb cross-engine dependency latency without costing a PSUM bank.
  - PSUM: ps0 [128,2,512] (block0), ps1 [128,3,512] (block1), ps2
    [128,2,512] (block2 pairs), psm [128,512] (paired m4, every other
    slice) = 8 banks; dequant is emitted after each round's evacs so
    the next slice's PSUM WAR clears early.
  - Output y rides DRAM as int8 (per-tensor scale, clip 4 sigma); evac is
    a plain fp32->int8 copy (HW rounds-to-nearest-even and saturates).
  - Input x: first 512 permuted features (block0 + block1 m0,m1) ride as
    int8, dequantized on-chip to fp16 (exact integers; scales fold into
    the fp16 weights): ACT g0,g1; DVE half of g2; GPSIMD g2-half + g3.
    Remaining 448 features ride fp16 and feed the PE directly. This
    balances DMA bytes (345 GB/s measured) against copy-engine rates
    (ACT ~0.86, DVE ~0.78, GPSIMD ~0.27 elem/ns measured).
  - Block2's five 64-wide m-components run at full PE width: (m0,m1) and
    (m2,m3) pair into 128 partitions with a block-diagonal W2; m4 of the
    window's first/second node-halves pair the same way (xbp/ybp tensors).
  - Dequant runs two slices ahead; loads on the SP HWDGE ring, stores on
    the ACT ring; windows [1024,3072x3,2048,212] keep head/tail short.
"""

import numpy as np

NCORES = 8
N_TOTAL = 100000
NSH = N_TOTAL // NCORES   # 12500 nodes per core
D = 960
MMW = 512                 # matmul slice width (= one fp32 PSUM bank)
PAD = 5                   # discardable pad matmuls per slice (DVFS keep-alive)

WINDOWS = [1024, 3072, 3072, 3072, 2048, 212]
assert sum(WINDOWS) == NSH and all(w % 2 == 0 for w in WINDOWS)
OFFS = np.concatenate([[0], np.cumsum(WINDOWS)[:-1]]).tolist()

CLIP_X = 4.0
CLIP_Y = 4.0
S_X = CLIP_X / 127.0
S_Y = CLIP_Y / 127.0

_BUILD_CACHE = {}


def _perm():
    p = list(range(256))
    for m in range(3):
        p += [256 + 3 * i + m for i in range(128)]
    for m in range(5):
        p += [640 + 5 * i + m for i in range(64)]
    return np.asarray(p, dtype=np.int64)

_PERM = _perm()


def _build_program():
    import concourse.bass as bass  # noqa: F401
    import concourse.bacc as bacc
    import concourse.mybir as mybir
    import concourse.tile as tile

    key = (MMW, tuple(WINDOWS), PAD, "v16")
    if key in _BUILD_CACHE:
        return _BUILD_CACHE[key]

    i8 = mybir.dt.int8
    f16 = mybir.dt.float16
    f32 = mybir.dt.float32

    nc = bacc.Bacc(
        "TRN2", target_bir_lowering=False, debug=False, enable_asserts=False
    )
    xa8 = nc.dram_tensor("xa8", [128, 4 * NSH], i8, kind="ExternalInput").ap()
    xa16 = nc.dram_tensor("xa16", [128, 3 * NSH], f16, kind="ExternalInput").ap()
    xbp = nc.dram_tensor("xbp", [128, NSH // 2], f16, kind="ExternalInput").ap()
    w0 = nc.dram_tensor("w0", [256, 256], f16, kind="ExternalInput").ap()
    w1q = nc.dram_tensor("w1q", [128, 128], f16, kind="ExternalInput").ap()
    w1f = nc.dram_tensor("w1f", [128, 128], f16, kind="ExternalInput").ap()
    w2d = nc.dram_tensor("w2d", [128, 128], f16, kind="ExternalInput").ap()
    ya = nc.dram_tensor("ya", [128, 7 * NSH], i8, kind="ExternalOutput").ap()
    ybp = nc.dram_tensor("ybp", [128, NSH // 2], i8, kind="ExternalOutput").ap()

    with tile.TileContext(nc) as tc:
        with (
            tc.tile_pool(name="sb", bufs=1) as spool,
            tc.tile_pool(name="ps", bufs=1, space="PSUM") as pspool,
        ):
            w0t0 = spool.tile([128, 256], f16, name="w0t0", tag="w0t0")
            nc.sync.dma_start(w0t0[:], w0[0:128, :])
            w0t1 = spool.tile([128, 256], f16, name="w0t1", tag="w0t1")
            nc.sync.dma_start(w0t1[:], w0[128:256, :])
            w1qt = spool.tile([128, 128], f16, name="w1qt", tag="w1qt")
            nc.sync.dma_start(w1qt[:], w1q[:, :])
            w1ft = spool.tile([128, 128], f16, name="w1ft", tag="w1ft")
            nc.sync.dma_start(w1ft[:], w1f[:, :])
            w2dt = spool.tile([128, 128], f16, name="w2dt", tag="w2dt")
            nc.sync.dma_start(w2dt[:], w2d[:, :])

            items = []
            for wi, (c0, sw) in enumerate(zip(OFFS, WINDOWS)):
                h = sw // 2
                xat8 = spool.tile([128, 4, sw], i8, name=f"xa8_{wi}",
                                  tag="xa8", bufs=2)
                nc.sync.dma_start(xat8[:], xa8[:, 4 * c0 : 4 * (c0 + sw)])
                xat16 = spool.tile([128, 3, sw], f16, name=f"xa16_{wi}",
                                   tag="xa16", bufs=2)
                nc.sync.dma_start(xat16[:], xa16[:, 3 * c0 : 3 * (c0 + sw)])
                xbt = spool.tile([128, h], f16, name=f"xb{wi}", tag="xb",
                                 bufs=2)
                nc.sync.dma_start(xbt[:], xbp[:, c0 // 2 : c0 // 2 + h])
                yat = spool.tile([128, 7, sw], i8, name=f"ya{wi}", tag="ya",
                                 bufs=2)
                ybt = spool.tile([128, h], i8, name=f"yb{wi}", tag="yb",
                                 bufs=2)

                slices = [
                    (i * MMW, min((i + 1) * MMW, sw))
                    for i in range((sw + MMW - 1) // MMW)
                ]
                m4s = [
                    (i * MMW, min((i + 1) * MMW, h))
                    for i in range((h + MMW - 1) // MMW)
                ]
                m4i = 0
                for si, (lo, hi) in enumerate(slices):
                    m4 = None
                    if (si % 2 == 1 or si == len(slices) - 1) and m4i < len(m4s):
                        m4 = m4s[m4i]
                        m4i += 1
                    last = si == len(slices) - 1
                    stores = (c0, sw, h) if last else None
                    items.append((lo, hi, xat8, xat16, xbt, yat, ybt, m4,
                                  stores))
                assert m4i == len(m4s)

            xfs = {}

            def deq(k):
                lo, hi, xat8, _, _, _, _, _, _ = items[k]
                n = hi - lo
                c = min(256, n)
                xf = spool.tile([128, 4, MMW], f16, name=f"xf{k}", tag="xf",
                                bufs=3)
                xfs[k] = xf
                nc.scalar.copy(xf[:, 0:2, 0:n], xat8[:, 0:2, lo:hi])
                nc.gpsimd.tensor_copy(xf[:, 3:4, 0:n], xat8[:, 3:4, lo:hi])
                nc.gpsimd.tensor_copy(xf[:, 2, 0:c], xat8[:, 2, lo : lo + c])
                if n > c:
                    nc.vector.tensor_copy(xf[:, 2, c:n], xat8[:, 2, lo + c : hi])

            def mm_evac(k):
                lo, hi, xat8, xat16, xbt, yat, ybt, m4, stores = items[k]
                n = hi - lo
                xf = xfs.pop(k)

                ps0 = pspool.tile([128, 2, MMW], f32, name=f"ps0_{k}",
                                  tag="ps0")
                ps1 = pspool.tile([128, 3, MMW], f32, name=f"ps1_{k}",
                                  tag="ps1")
                ps2 = pspool.tile([128, 2, MMW], f32, name=f"ps2_{k}",
                                  tag="ps2")

                # pads: overwritten by block0's first group below (WAW on PE)
                for p in range(PAD):
                    nc.tensor.matmul(
                        ps0[:, 0, 0:n], w2dt[:], xat16[:, p % 3, lo:hi],
                        start=True, stop=True,
                    )

                # block0: 256x256 = 2 out-blocks x 2 K-groups
                for ob in range(2):
                    oc = slice(128 * ob, 128 * (ob + 1))
                    nc.tensor.matmul(
                        ps0[:, ob, 0:n], w0t0[:, oc], xf[:, 0, 0:n],
                        start=True, stop=False,
                    )
                    nc.tensor.matmul(
                        ps0[:, ob, 0:n], w0t1[:, oc], xf[:, 1, 0:n],
                        start=False, stop=True,
                    )
                nc.scalar.copy(yat[:, 0:2, lo:hi], ps0[:, :, 0:n])

                # block1: m0,m1 (int8 path), m2 (fp16 direct)
                for m in range(2):
                    nc.tensor.matmul(
                        ps1[:, m, 0:n], w1qt[:], xf[:, 2 + m, 0:n],
                        start=True, stop=True,
                    )
                nc.tensor.matmul(
                    ps1[:, 2, 0:n], w1ft[:], xat16[:, 0, lo:hi],
                    start=True, stop=True,
                )
                nc.vector.tensor_copy(yat[:, 2:5, lo:hi], ps1[:, :, 0:n])

                # block2 (m0,m1),(m2,m3) pairs + paired m4 (all fp16 direct)
                for g in range(2):
                    nc.tensor.matmul(
                        ps2[:, g, 0:n], w2dt[:], xat16[:, 1 + g, lo:hi],
                        start=True, stop=True,
                    )
                nc.scalar.copy(yat[:, 5:6, lo:hi], ps2[:, 0:1, 0:n])
                nc.vector.tensor_copy(yat[:, 6:7, lo:hi], ps2[:, 1:2, 0:n])
                if m4 is not None:
                    ko, khi = m4
                    psm = pspool.tile([128, MMW], f32, name=f"psm_{k}",
                                      tag="psm")
                    nc.tensor.matmul(
                        psm[:, 0 : khi - ko], w2dt[:], xbt[:, ko:khi],
                        start=True, stop=True,
                    )
                    nc.scalar.copy(ybt[:, ko:khi], psm[:, 0 : khi - ko])

                if stores is not None:
                    c0, sw, h = stores
                    nc.scalar.dma_start(ya[:, 7 * c0 : 7 * (c0 + sw)], yat[:])
                    nc.scalar.dma_start(ybp[:, c0 // 2 : c0 // 2 + h], ybt[:])

            NK = len(items)
            deq(0)
            deq(1)
            for k in range(NK):
                mm_evac(k)
                if k + 2 < NK:
                    deq(k + 2)

    nc.compile()
    _BUILD_CACHE[key] = nc
    return nc


TRACE = False
LAST_RESULT = None


def kernel(x, W0, W1, W2):
    from concourse import bass_utils

    nc = _build_program()

    # weights: fold 1/sqrt(mul), input scale (int8 paths) and output scale
    w0s = (np.asarray(W0, np.float32) * (S_X / (16.0 * S_Y))).astype(np.float16)
    w1qs = (np.asarray(W1, np.float32) * (S_X / (np.sqrt(128.0) * S_Y))
            ).astype(np.float16)
    w1fs = (np.asarray(W1, np.float32) * (1.0 / (np.sqrt(128.0) * S_Y))
            ).astype(np.float16)
    w2 = (np.asarray(W2, np.float32) * (1.0 / (8.0 * S_Y))).astype(np.float16)
    w2dv = np.zeros((128, 128), dtype=np.float16)
    w2dv[0:64, 0:64] = w2
    w2dv[64:128, 64:128] = w2

    A = np.asarray(x)[:, _PERM].reshape(NCORES, NSH, D)
    A8 = np.clip(np.rint(A[:, :, :512] * (1.0 / S_X)), -127, 127
                 ).astype(np.int8)
    A16 = A[:, :, 512:896].astype(np.float16)
    AB = A[:, :, 896:].astype(np.float16)

    blocks8, blocks16, bblocks = [], [], []
    for c0, sw in zip(OFFS, WINDOWS):
        b8 = A8[:, c0 : c0 + sw].reshape(NCORES, sw, 4, 128)
        blocks8.append(b8.transpose(0, 3, 2, 1).reshape(NCORES, 128, 4 * sw))
        b16 = A16[:, c0 : c0 + sw].reshape(NCORES, sw, 3, 128)
        blocks16.append(b16.transpose(0, 3, 2, 1).reshape(NCORES, 128, 3 * sw))
        F = AB[:, c0 : c0 + sw]                          # [C, sw, 64]
        h = sw // 2
        bblocks.append(np.concatenate(
            [F[:, :h].transpose(0, 2, 1), F[:, h:].transpose(0, 2, 1)], axis=1
        ))                                               # [C, 128, h]
    xa8_all = np.ascontiguousarray(np.concatenate(blocks8, axis=2))
    xa16_all = np.ascontiguousarray(np.concatenate(blocks16, axis=2))
    xb_all = np.ascontiguousarray(np.concatenate(bblocks, axis=2))

    in_maps = []
    for c in range(NCORES):
        in_maps.append({
            "xa8": xa8_all[c], "xa16": xa16_all[c], "xbp": xb_all[c],
            "w0": w0s, "w1q": w1qs, "w1f": w1fs, "w2d": w2dv,
        })

    res = bass_utils.run_bass_kernel_spmd(
        nc, in_maps, core_ids=list(range(NCORES)), trace=TRACE
    )
    global LAST_RESULT
    LAST_RESULT = res

    out = np.empty((N_TOTAL, D), dtype=np.float32)
    Yp = np.empty((NCORES, NSH, D), dtype=np.float32)
    for c in range(NCORES):
        yac = res.results[c]["ya"]    # [128, 7*NSH] int8
        ybc = res.results[c]["ybp"]   # [128, NSH//2] int8
        for c0, sw in zip(OFFS, WINDOWS):
            blk = yac[:, 7 * c0 : 7 * (c0 + sw)].reshape(128, 7, sw)
            Yp[c, c0 : c0 + sw, :896] = (
                blk.transpose(2, 1, 0).reshape(sw, 896).astype(np.float32)
            )
            h = sw // 2
            yb = ybc[:, c0 // 2 : c0 // 2 + h]
            Yp[c, c0 : c0 + h, 896:] = yb[0:64].T.astype(np.float32)
            Yp[c, c0 + h : c0 + sw, 896:] = yb[64:128].T.astype(np.float32)
    Yp *= S_Y
    out[:, _PERM] = Yp.reshape(N_TOTAL, D)
    return out


# revision 10
# speedup vs baseline: 1.2665x; 1.0147x over previous
"""IrrepsLinear Trainium2 kernel: y = per-irrep-block x @ W / sqrt(mul).

Irreps layout: 256x0e + 128x1o + 64x2e -> blocks of width 256*1, 128*3, 64*5.
Data-parallel over 8 NeuronCores: each core gets 12500 nodes.

v15 strategy (hybrid int8/fp16 in, int8 out, latency-tuned at MMW=512):
  - MMW=512 keeps LDWEIGHTS hidden behind matmuls (at 256 it serializes
    and halves PE throughput). The PE sustains ~2.24 GHz when matmuls
    issue near back-to-back; 3 discardable pad matmuls at each slice
    start (written into ps0's bank, overwritten by the real block0 group)
    absorb cross-engine dependency latency without costing a PSUM bank.
  - PSUM: ps0 [128,2,512] (block0), ps1 [128,3,512] (block1), ps2
    [128,2,512] (block2 pairs), psm [128,512] (paired m4, every other
    slice) = 8 banks; dequant is emitted after each round's evacs so
    the next slice's PSUM WAR clears early.
  - Output y rides DRAM as int8 (per-tensor scale, clip 4 sigma); evac is
    a plain fp32->int8 copy (HW rounds-to-nearest-even and saturates).
  - Input x: first 512 permuted features (block0 + block1 m0,m1) ride as
    int8, dequantized on-chip to fp16 (exact integers; scales fold into
    the fp16 weights): ACT g0,g1; DVE half of g2; GPSIMD g2-half + g3.
    Remaining 448 features ride fp16 and feed the PE directly. This
    balances DMA bytes (345 GB/s measured) against copy-engine rates
    (ACT ~0.86, DVE ~0.78, GPSIMD ~0.27 elem/ns measured).
  - Block2's five 64-wide m-components run at full PE width: (m0,m1) and
    (m2,m3) pair into 128 partitions with a block-diagonal W2; m4 of the
    window's first/second node-halves pair the same way (xbp/ybp tensors).
  - Dequant runs two slices ahead; loads on the SP HWDGE ring, stores on
    the ACT ring; windows [1024,3072x3,2048,212] keep head/tail short.
"""

import numpy as np

NCORES = 8
N_TOTAL = 100000
NSH = N_TOTAL // NCORES   # 12500 nodes per core
D = 960
MMW = 512                 # matmul slice width (= one fp32 PSUM bank)
PAD = 5                   # discardable pad matmuls per slice (DVFS keep-alive)

WINDOWS = [512] + [1024] * 11 + [512, 212]   # load windows
assert sum(WINDOWS) == NSH and all(w % 2 == 0 for w in WINDOWS)
OFFS = np.concatenate([[0], np.cumsum(WINDOWS)[:-1]]).tolist()
# store groups: slices of WINDOWS list sharing one output tile/store DMA
SGROUPS = [(0, 2), (2, 5), (5, 8), (8, 11), (11, 13), (13, 14)]
SOFFS = [sum(WINDOWS[:a]) for a, b in SGROUPS]
SSIZES = [sum(WINDOWS[a:b]) for a, b in SGROUPS]

CLIP_X = 4.0
CLIP_Y = 4.0
S_X = CLIP_X / 127.0
S_Y = CLIP_Y / 127.0

_BUILD_CACHE = {}


def _perm():
    p = list(range(256))
    for m in range(3):
        p += [256 + 3 * i + m for i in range(128)]
    for m in range(5):
        p += [640 + 5 * i + m for i in range(64)]
    return np.asarray(p, dtype=np.int64)

_PERM = _perm()


def _build_program():
    import concourse.bass as bass  # noqa: F401
    import concourse.bacc as bacc
    import concourse.mybir as mybir
    import concourse.tile as tile

    key = (MMW, tuple(WINDOWS), PAD, "v17")
    if key in _BUILD_CACHE:
        return _BUILD_CACHE[key]

    i8 = mybir.dt.int8
    f16 = mybir.dt.float16
    f32 = mybir.dt.float32

    nc = bacc.Bacc(
        "TRN2", target_bir_lowering=False, debug=False, enable_asserts=False
    )
    xa8 = nc.dram_tensor("xa8", [128, 4 * NSH], i8, kind="ExternalInput").ap()
    xa16 = nc.dram_tensor("xa16", [128, 3 * NSH], f16, kind="ExternalInput").ap()
    xbp = nc.dram_tensor("xbp", [128, NSH // 2], f16, kind="ExternalInput").ap()
    w0 = nc.dram_tensor("w0", [256, 256], f16, kind="ExternalInput").ap()
    w1q = nc.dram_tensor("w1q", [128, 128], f16, kind="ExternalInput").ap()
    w1f = nc.dram_tensor("w1f", [128, 128], f16, kind="ExternalInput").ap()
    w2d = nc.dram_tensor("w2d", [128, 128], f16, kind="ExternalInput").ap()
    ya = nc.dram_tensor("ya", [128, 7 * NSH], i8, kind="ExternalOutput").ap()
    ybp = nc.dram_tensor("ybp", [128, NSH // 2], i8, kind="ExternalOutput").ap()

    with tile.TileContext(nc) as tc:
        with (
            tc.tile_pool(name="sb", bufs=1) as spool,
            tc.tile_pool(name="ps", bufs=1, space="PSUM") as pspool,
        ):
            w0t0 = spool.tile([128, 256], f16, name="w0t0", tag="w0t0")
            nc.sync.dma_start(w0t0[:], w0[0:128, :])
            w0t1 = spool.tile([128, 256], f16, name="w0t1", tag="w0t1")
            nc.sync.dma_start(w0t1[:], w0[128:256, :])
            w1qt = spool.tile([128, 128], f16, name="w1qt", tag="w1qt")
            nc.sync.dma_start(w1qt[:], w1q[:, :])
            w1ft = spool.tile([128, 128], f16, name="w1ft", tag="w1ft")
            nc.sync.dma_start(w1ft[:], w1f[:, :])
            w2dt = spool.tile([128, 128], f16, name="w2dt", tag="w2dt")
            nc.sync.dma_start(w2dt[:], w2d[:, :])

            # store tiles, one per store group
            stiles = []
            for gi, ((a, b), sc0, ssw) in enumerate(zip(SGROUPS, SOFFS, SSIZES)):
                yat = spool.tile([128, 7, ssw], i8, name=f"ya{gi}", tag="ya",
                                 bufs=2)
                ybt = spool.tile([128, ssw // 2], i8, name=f"yb{gi}",
                                 tag="yb", bufs=2)
                stiles.append((yat, ybt))

            items = []
            for wi, (c0, sw) in enumerate(zip(OFFS, WINDOWS)):
                h = sw // 2
                gi = next(i for i, (a, b) in enumerate(SGROUPS)
                          if a <= wi < b)
                yat, ybt = stiles[gi]
                yoff = c0 - SOFFS[gi]          # node offset inside store tile
                xat8 = spool.tile([128, 4, sw], i8, name=f"xa8_{wi}",
                                  tag="xa8", bufs=3)
                nc.sync.dma_start(xat8[:], xa8[:, 4 * c0 : 4 * (c0 + sw)])
                xat16 = spool.tile([128, 3, sw], f16, name=f"xa16_{wi}",
                                   tag="xa16", bufs=3)
                nc.sync.dma_start(xat16[:], xa16[:, 3 * c0 : 3 * (c0 + sw)])
                xbt = spool.tile([128, h], f16, name=f"xb{wi}", tag="xb",
                                 bufs=3)
                nc.sync.dma_start(xbt[:], xbp[:, c0 // 2 : c0 // 2 + h])

                slices = [
                    (i * MMW, min((i + 1) * MMW, sw))
                    for i in range((sw + MMW - 1) // MMW)
                ]
                m4s = [
                    (i * MMW, min((i + 1) * MMW, h))
                    for i in range((h + MMW - 1) // MMW)
                ]
                m4i = 0
                for si, (lo, hi) in enumerate(slices):
                    m4 = None
                    if (si % 2 == 1 or si == len(slices) - 1) and m4i < len(m4s):
                        m4 = m4s[m4i]
                        m4i += 1
                    stores = None
                    if wi == SGROUPS[gi][1] - 1 and si == len(slices) - 1:
                        stores = (SOFFS[gi], SSIZES[gi])
                    items.append((lo, hi, xat8, xat16, xbt, yat, ybt, m4,
                                  stores, yoff))
                assert m4i == len(m4s)

            xfs = {}

            def deq(k):
                lo, hi, xat8 = items[k][:3]
                n = hi - lo
                c = min(256, n)
                xf = spool.tile([128, 4, MMW], f16, name=f"xf{k}", tag="xf",
                                bufs=4)
                xfs[k] = xf
                nc.scalar.copy(xf[:, 0:2, 0:n], xat8[:, 0:2, lo:hi])
                nc.gpsimd.tensor_copy(xf[:, 3:4, 0:n], xat8[:, 3:4, lo:hi])
                nc.gpsimd.tensor_copy(xf[:, 2, 0:c], xat8[:, 2, lo : lo + c])
                if n > c:
                    nc.vector.tensor_copy(xf[:, 2, c:n], xat8[:, 2, lo + c : hi])

            def mm_evac(k):
                lo, hi, xat8, xat16, xbt, yat, ybt, m4, stores, yoff = items[k]
                n = hi - lo
                slo, shi = yoff + lo, yoff + hi
                xf = xfs.pop(k)

                ps0 = pspool.tile([128, 2, MMW], f32, name=f"ps0_{k}",
                                  tag="ps0")
                ps1 = pspool.tile([128, 3, MMW], f32, name=f"ps1_{k}",
                                  tag="ps1")
                ps2 = pspool.tile([128, 2, MMW], f32, name=f"ps2_{k}",
                                  tag="ps2")

                # pads: overwritten by block0's first group below (WAW on PE)
                for p in range(PAD):
                    nc.tensor.matmul(
                        ps0[:, 0, 0:n], w2dt[:], xat16[:, p % 3, lo:hi],
                        start=True, stop=True,
                    )

                # block0: 256x256 = 2 out-blocks x 2 K-groups
                for ob in range(2):
                    oc = slice(128 * ob, 128 * (ob + 1))
                    nc.tensor.matmul(
                        ps0[:, ob, 0:n], w0t0[:, oc], xf[:, 0, 0:n],
                        start=True, stop=False,
                    )
                    nc.tensor.matmul(
                        ps0[:, ob, 0:n], w0t1[:, oc], xf[:, 1, 0:n],
                        start=False, stop=True,
                    )
                nc.scalar.copy(yat[:, 0:2, slo:shi], ps0[:, :, 0:n])

                # block1: m0,m1 (int8 path), m2 (fp16 direct)
                for m in range(2):
                    nc.tensor.matmul(
                        ps1[:, m, 0:n], w1qt[:], xf[:, 2 + m, 0:n],
                        start=True, stop=True,
                    )
                nc.tensor.matmul(
                    ps1[:, 2, 0:n], w1ft[:], xat16[:, 0, lo:hi],
                    start=True, stop=True,
                )
                nc.vector.tensor_copy(yat[:, 2:5, slo:shi], ps1[:, :, 0:n])

                # block2 (m0,m1),(m2,m3) pairs + paired m4 (all fp16 direct)
                for g in range(2):
                    nc.tensor.matmul(
                        ps2[:, g, 0:n], w2dt[:], xat16[:, 1 + g, lo:hi],
                        start=True, stop=True,
                    )
                nc.scalar.copy(yat[:, 5:6, slo:shi], ps2[:, 0:1, 0:n])
                nc.vector.tensor_copy(yat[:, 6:7, slo:shi], ps2[:, 1:2, 0:n])
                if m4 is not None:
                    ko, khi = m4
                    bo = yoff // 2
                    psm = pspool.tile([128, MMW], f32, name=f"psm_{k}",
                                      tag="psm")
                    nc.tensor.matmul(
                        psm[:, 0 : khi - ko], w2dt[:], xbt[:, ko:khi],
                        start=True, stop=True,
                    )
                    nc.scalar.copy(ybt[:, bo + ko : bo + khi],
                                   psm[:, 0 : khi - ko])

                if stores is not None:
                    sc0, ssw = stores
                    nc.scalar.dma_start(ya[:, 7 * sc0 : 7 * (sc0 + ssw)],
                                        yat[:])
                    nc.scalar.dma_start(
                        ybp[:, sc0 // 2 : (sc0 + ssw) // 2], ybt[:])

            NK = len(items)
            deq(0)
            deq(1)
            deq(2)
            for k in range(NK):
                mm_evac(k)
                if k + 3 < NK:
                    deq(k + 3)

    nc.compile()
    _BUILD_CACHE[key] = nc
    return nc


TRACE = False
LAST_RESULT = None


def kernel(x, W0, W1, W2):
    from concourse import bass_utils

    nc = _build_program()

    # weights: fold 1/sqrt(mul), input scale (int8 paths) and output scale
    w0s = (np.asarray(W0, np.float32) * (S_X / (16.0 * S_Y))).astype(np.float16)
    w1qs = (np.asarray(W1, np.float32) * (S_X / (np.sqrt(128.0) * S_Y))
            ).astype(np.float16)
    w1fs = (np.asarray(W1, np.float32) * (1.0 / (np.sqrt(128.0) * S_Y))
            ).astype(np.float16)
    w2 = (np.asarray(W2, np.float32) * (1.0 / (8.0 * S_Y))).astype(np.float16)
    w2dv = np.zeros((128, 128), dtype=np.float16)
    w2dv[0:64, 0:64] = w2
    w2dv[64:128, 64:128] = w2

    A = np.asarray(x)[:, _PERM].reshape(NCORES, NSH, D)
    A8 = np.clip(np.rint(A[:, :, :512] * (1.0 / S_X)), -127, 127
                 ).astype(np.int8)
    A16 = A[:, :, 512:896].astype(np.float16)
    AB = A[:, :, 896:].astype(np.float16)

    blocks8, blocks16, bblocks = [], [], []
    for c0, sw in zip(OFFS, WINDOWS):
        b8 = A8[:, c0 : c0 + sw].reshape(NCORES, sw, 4, 128)
        blocks8.append(b8.transpose(0, 3, 2, 1).reshape(NCORES, 128, 4 * sw))
        b16 = A16[:, c0 : c0 + sw].reshape(NCORES, sw, 3, 128)
        blocks16.append(b16.transpose(0, 3, 2, 1).reshape(NCORES, 128, 3 * sw))
        F = AB[:, c0 : c0 + sw]                          # [C, sw, 64]
        h = sw // 2
        bblocks.append(np.concatenate(
            [F[:, :h].transpose(0, 2, 1), F[:, h:].transpose(0, 2, 1)], axis=1
        ))                                               # [C, 128, h]
    xa8_all = np.ascontiguousarray(np.concatenate(blocks8, axis=2))
    xa16_all = np.ascontiguousarray(np.concatenate(blocks16, axis=2))
    xb_all = np.ascontiguousarray(np.concatenate(bblocks, axis=2))

    in_maps = []
    for c in range(NCORES):
        in_maps.append({
            "xa8": xa8_all[c], "xa16": xa16_all[c], "xbp": xb_all[c],
            "w0": w0s, "w1q": w1qs, "w1f": w1fs, "w2d": w2dv,
        })

    res = bass_utils.run_bass_kernel_spmd(
        nc, in_maps, core_ids=list(range(NCORES)), trace=TRACE
    )
    global LAST_RESULT
    LAST_RESULT = res

    out = np.empty((N_TOTAL, D), dtype=np.float32)
    Yp = np.empty((NCORES, NSH, D), dtype=np.float32)
    for c in range(NCORES):
        yac = res.results[c]["ya"]    # [128, 7*NSH] int8
        ybc = res.results[c]["ybp"]   # [128, NSH//2] int8
        for sc0, ssw in zip(SOFFS, SSIZES):
            blk = yac[:, 7 * sc0 : 7 * (sc0 + ssw)].reshape(128, 7, ssw)
            Yp[c, sc0 : sc0 + ssw, :896] = (
                blk.transpose(2, 1, 0).reshape(ssw, 896).astype(np.float32)
            )
        for c0, sw in zip(OFFS, WINDOWS):
            h = sw // 2
            yb = ybc[:, c0 // 2 : c0 // 2 + h]
            Yp[c, c0 : c0 + h, 896:] = yb[0:64].T.astype(np.float32)
            Yp[c, c0 + h : c0 + sw, 896:] = yb[64:128].T.astype(np.float32)
    Yp *= S_Y
    out[:, _PERM] = Yp.reshape(N_TOTAL, D)
    return out
